# revision 44
# baseline (speedup 1.0000x reference)
"""Multi-head causal attention (B=4, C=2048, E=1024, H=16, D=64) on 8 TRN2 cores.

Sharding: batch x head-group (4 x 2). Core c handles batch c//2 and heads
(c%2)*8 .. (c%2)*8+8.  Each core computes a partial output

    Y_c = Attn(x_b; heads hg) @ W_o[hg rows]        (shape [C, E])

and the host sums the two partials per batch.

Design (537us fp32r baseline -> 277us):
  * all matmul operands bf16 (PSUM accum stays fp32) - same PE streaming
    rate as fp32r at N=512 but half the DMA/SBUF and fast weight loads.
  * softly-pipelined schedule: attention "unit steps" (S^T pair matmul ->
    exp on ACT -> P@V), which feed the ACT-bound softmax exp, are woven
    between projection / output-projection chains at a ~3:1 quota so the
    PE and ACT engines both stay >90% busy.  Unit (j, g) unblocks as soon
    as its c-slice's V chains and the K/Q chains for head pair g are
    emitted (emission order IS program order - a step emitted before its
    producer chain would read stale SBUF).  ph3 chains are held back to
    fill the PE during the exp-heavy attention tail.
  * ~7us of dummy warm-up matmuls during the input-DMA wait plus dense
    back-to-back scheduling keep the HAM clock gate at K=8/8 (the
    baseline spent 320us at half clock).
  * input DMAs ordered so everything kqv(0) reads lands first (the
    framework lowers DMA deps to queue-position waits, so a late wv
    stalled the first V chains 15us in an earlier version).
  * causal straddle tiles are column-trimmed: S / exp / P@V only touch
    q >= w; the 128x128 diagonal straddle gets a 0/1 upper-triangular
    multiply on DVE (bf16, both heads in one op).  No mask memsets.
  * V carries 64 ones-columns so the P@V matmul yields the softmax
    denominator replicated on partitions 64..127 of the same PSUM bank;
    1/denom = bit-trick seed (XOR + int32 add; DVE arith ALU ops compute
    in fp32 even on u32 APs, so MAGIC-bits(d) is built as ~bits+(MAGIC+1))
    plus one Newton-Raphson step fused into the normalizing multiplies -
    4 full-width DVE ops, no 1-partition reciprocal (3.3us each in the
    baseline), no broadcast matmul.
  * per-kk-tile issue order S(kkt) ... PV(kkt-1) keeps a spare exp queued
    for ACT while the PE never sits directly behind an exp-dependent PV.
"""

import sys

if "/opt/trn_rl_repo" not in sys.path:
    sys.path.insert(0, "/opt/trn_rl_repo")

import math

import numpy as np

B, C, E, H, D = 4, 2048, 1024, 16, 64
NCORES = 8
P = 128
CS = 512  # q-slice width


def build_module(C=C, E=E, HL=H // 2, D=D, n_devices=NCORES):
    """Build the SPMD Bass module for one core's shard."""
    from contextlib import ExitStack

    import concourse.bass as bass
    import concourse.mybir as mybir
    import concourse.tile as tile

    F32 = mybir.dt.float32
    BF16 = mybir.dt.bfloat16
    U32 = mybir.dt.uint32
    I32 = mybir.dt.int32
    Exp = mybir.ActivationFunctionType.Exp
    MUL = mybir.AluOpType.mult
    ADD = mybir.AluOpType.add
    SUB = mybir.AluOpType.subtract
    XOR = mybir.AluOpType.bitwise_xor
    RCP_MAGIC = 0x7EF127EA

    ET = E // P          # e-tiles (8)
    JT = HL * D // P     # j-tiles / head pairs (4)
    NJ = C // CS         # q-slices (4)
    CT = C // P          # c-tiles (16)
    KPJ = CS // P        # kk-tiles per q-slice (4)
    scale = 1.0 / math.sqrt(D)

    nc = bass.Bass(
        "TRN2", target_bir_lowering=False, debug=False, num_devices=n_devices
    )

    xT = nc.dram_tensor("xT", [P, NJ, ET, CS], BF16, kind="ExternalInput").ap()
    wq_d = nc.dram_tensor("wq", [P, ET, HL * D], BF16, kind="ExternalInput").ap()
    wk_d = nc.dram_tensor("wk", [P, ET, HL * D], BF16, kind="ExternalInput").ap()
    wv_d = nc.dram_tensor("wv", [P, ET, HL * D], BF16, kind="ExternalInput").ap()
    wo_d = nc.dram_tensor("wo", [P, JT, E], BF16, kind="ExternalInput").ap()
    tm_d = nc.dram_tensor("tm", [P, 2, P], BF16, kind="ExternalInput").ap()
    y_d = nc.dram_tensor("y", [CT, P, E], F32, kind="ExternalOutput").ap()

    with tile.TileContext(nc) as tc:
        with ExitStack() as ctx:
            pA = ctx.enter_context(tc.tile_pool(name="pA", bufs=1))
            pE = ctx.enter_context(tc.tile_pool(name="pE", bufs=8))
            pN = ctx.enter_context(tc.tile_pool(name="pN", bufs=6))
            pY = ctx.enter_context(tc.tile_pool(name="pY", bufs=3))
            psS = ctx.enter_context(tc.tile_pool(name="psS", bufs=2, space="PSUM"))
            psPV = ctx.enter_context(tc.tile_pool(name="psPV", bufs=2, space="PSUM"))
            psMM = ctx.enter_context(tc.tile_pool(name="psMM", bufs=2, space="PSUM"))

            qt = pA.tile([P, JT, C], BF16, tag="qt")
            kt = pA.tile([P, JT, C], BF16, tag="kt")
            v = pA.tile([P, CT, HL, 2 * D], BF16, tag="v")
            hdt = pA.tile([P, JT, C], BF16, tag="hdt")
            wo = pA.tile([P, JT, E], BF16, tag="wo")
            tm = pA.tile([P, 2, P], BF16, tag="tm")
            xs = pA.tile([P, NJ, ET, CS], BF16, tag="xs")
            wk = pA.tile([P, ET, HL * D], BF16, tag="wk")
            wq = pA.tile([P, ET, HL * D], BF16, tag="wq")
            wv = pA.tile([P, ET, HL * D], BF16, tag="wv")

            # DMA order matters: everything kqv(0) touches must land first
            # (all input DMAs serialize on one hardware queue, and the
            # framework lowers dependencies to queue-position waits).
            nc.sync.dma_start(xs[:, 0], xT[:, 0])
            nc.sync.dma_start(wv[:], wv_d)
            nc.sync.dma_start(wq[:], wq_d)
            nc.sync.dma_start(wk[:], wk_d)
            nc.sync.dma_start(tm[:], tm_d)
            for cs in range(1, NJ):
                nc.sync.dma_start(xs[:, cs], xT[:, cs])
            nc.sync.dma_start(wo[:], wo_d)
            # PE warm-up: ~7us of dummy matmuls during the input-DMA wait so
            # the HAM clock gate reaches K=8/8 before real work starts.
            # wrm memset goes FIRST on the DVE queue (before the big strided
            # v-ones memset) and the matmuls alternate two PSUM banks so the
            # write-after-write chains interleave instead of serializing.
            wrm = pA.tile([P, CS], BF16, tag="wrm")
            nc.vector.memset(wrm[:], 0.25)
            wps = psS.tile([P, 2, CS], F32, tag="s")
            for i in range(20):
                nc.tensor.matmul(
                    wps[:, i % 2, 0 : 2 * P],
                    wrm[:, 0:P],
                    wrm[:, 0 : 2 * P],
                    start=True,
                    stop=True,
                )
            # dummy read so the warm-up tile releases its psS pool slot
            # before the first attention S matmuls need it
            wrd = pY.tile([P, CS], F32, tag="ysb")
            nc.vector.tensor_copy(wrd[:], wps[:, 0, :])

            # ones-columns: P@V row block 64:128 becomes the softmax denom
            nc.gpsimd.memset(v[:, :, :, D : 2 * D], 1.0)

            # ---------- emission generators (yield = one schedulable chunk)
            def kqv_chains(cs):
                """K^T, Q^T, V projections for c-slice cs; 12 chains."""
                csl = slice(cs * CS, (cs + 1) * CS)
                # early segments have an idle ACT engine (little exp work
                # yet); give it the PSUM evictions there to unclog DVE.
                # K/Q chains interleave per head-pair jt so attention unit
                # (cs, g) unblocks after only 2g+2 chains of this window.
                act_copy = cs <= 1
                for c4 in range(KPJ):
                    ct = cs * KPJ + c4
                    ps = psMM.tile([P, HL, D], F32, tag="mm")
                    for et in range(ET):
                        nc.tensor.matmul(
                            ps[:],
                            xs[:, cs, et, c4 * P : (c4 + 1) * P],
                            wv[:, et, :],
                            start=(et == 0),
                            stop=(et == ET - 1),
                        )
                    if act_copy:
                        nc.scalar.copy(v[:, ct, :, 0:D], ps[:])
                    else:
                        nc.vector.tensor_copy(v[:, ct, :, 0:D], ps[:])
                    yield 1
                for jt in range(JT):
                    for w_sb, out_t in ((wk, kt), (wq, qt)):
                        ps = psMM.tile([P, CS], F32, tag="mm")
                        for et in range(ET):
                            nc.tensor.matmul(
                                ps[:],
                                w_sb[:, et, jt * P : (jt + 1) * P],
                                xs[:, cs, et, :],
                                start=(et == 0),
                                stop=(et == ET - 1),
                            )
                        if act_copy and w_sb is wq:
                            nc.scalar.copy(out_t[:, jt, csl], ps[:])
                        else:
                            nc.vector.tensor_copy(out_t[:, jt, csl], ps[:])
                        yield 1

            def unit_steps(j, g):
                """One attention unit (head pair g, q-slice j); yields per
                kk-tile.  Issue order keeps S a step ahead of PV so ACT
                always has an exp queued."""
                jsl = slice(j * CS, (j + 1) * CS)
                nkt = (j + 1) * KPJ
                pv = [
                    psPV.tile([P, CS], F32, tag="pv", name=f"pv{h}")
                    for h in range(2)
                ]
                pend = None  # (kkt, lo, e) awaiting its PV matmuls
                for kkt in range(nkt):
                    w = kkt * P - j * CS
                    lo = max(w, 0)
                    ksl = slice(kkt * P, (kkt + 1) * P)
                    qsl = slice(j * CS + lo, (j + 1) * CS)
                    s_ps = psS.tile([P, 2, CS], F32, tag="s")
                    for half, base in ((0, 0), (1, 64)):
                        nc.tensor.matmul(
                            s_ps[:, half, lo:],
                            kt[base : base + 64, g, ksl],
                            qt[base : base + 64, g, qsl],
                            start=True,
                            stop=True,
                            tile_position=(base, 0),
                        )
                    e = pE.tile([P, 2, CS], BF16, tag="e")
                    nc.scalar.activation(
                        e[:, :, lo:], s_ps[:, :, lo:], Exp, scale=scale
                    )
                    if w >= 0:
                        blk = e[:, :, w : w + P]
                        nc.vector.tensor_tensor(blk, blk, tm[:], MUL)
                    if pend is not None:
                        pk, plo, pe_sb = pend
                        for half in range(2):
                            nc.tensor.matmul(
                                pv[half][:, plo:],
                                v[:, pk, 2 * g + half, :],
                                pe_sb[:, half, plo:],
                                start=(pk == 0),
                                stop=False,
                                skip_group_check=True,
                            )
                    pend = (kkt, lo, e)
                    yield 1
                pk, plo, pe_sb = pend
                for half in range(2):
                    nc.tensor.matmul(
                        pv[half][:, plo:],
                        v[:, pk, 2 * g + half, :],
                        pe_sb[:, half, plo:],
                        start=(pk == 0),
                        stop=True,
                        skip_group_check=True,
                    )
                # normalization: hd / denom with denom replicated on
                # partitions 64:127 (V ones-columns).  1/denom via magic
                # seed + one Newton-Raphson step, sign folded so only
                # subtract/mult ALU ops are needed:
                #   s = seed ~= 1/d;  t = d*s;  u = -hd*s
                #   out = (t-2)*u = (2-t)*s*hd ~= hd/d
                for half in range(2):
                    sd = pN.tile([64, CS], F32, tag="sd")
                    t = pN.tile([64, CS], F32, tag="t")
                    u = pN.tile([64, CS], F32, tag="u")
                    # seed bits = MAGIC - bits(d) == ~bits(d) + (MAGIC+1).
                    # DVE arith ALU ops compute in fp32 even on u32 APs, so
                    # use bitwise XOR (exact) + int32 add (the +-64ulp
                    # int-as-float rounding is crushed by the NR step).
                    nc.vector.tensor_scalar(
                        sd[:].bitcast(U32),
                        pv[half][64:128, :].bitcast(U32),
                        0xFFFFFFFF,
                        None,
                        XOR,
                    )
                    nc.vector.tensor_scalar(
                        sd[:].bitcast(I32),
                        sd[:].bitcast(I32),
                        RCP_MAGIC + 1,
                        None,
                        ADD,
                    )
                    nc.vector.tensor_tensor(
                        t[:], pv[half][64:128, :], sd[:], MUL
                    )
                    nc.vector.scalar_tensor_tensor(
                        u[:], pv[half][0:64, :], -1.0, sd[:], MUL, MUL
                    )
                    nc.vector.scalar_tensor_tensor(
                        hdt[64 * half : 64 * half + 64, g, jsl],
                        t[:],
                        2.0,
                        u[:],
                        SUB,
                        MUL,
                    )
                yield 1

            def ph3_chains(j):
                """Output projection for q-slice j's c-tiles; 8 chains.
                Evictions alternate DVE/ACT to spread engine load."""
                FS = min(CS, E)
                for c4 in range(KPJ):
                    ct = j * KPJ + c4
                    for fs in range(E // FS):
                        fsl = slice(fs * FS, (fs + 1) * FS)
                        ps = psMM.tile([P, FS], F32, tag="mm")
                        for jt in range(JT):
                            nc.tensor.matmul(
                                ps[:],
                                hdt[:, jt, ct * P : (ct + 1) * P],
                                wo[:, jt, fsl],
                                start=(jt == 0),
                                stop=(jt == JT - 1),
                            )
                        ysb = pY.tile([P, FS], F32, tag="ysb")
                        if (c4 + fs) % 2 == 0:
                            nc.vector.tensor_copy(ysb[:], ps[:])
                        else:
                            nc.scalar.copy(ysb[:], ps[:])
                        nc.sync.dma_start(y_d[ct, :, fsl], ysb[:])
                        yield 1

            # ---------- software-pipelined schedule.
            # Attention unit-steps (which feed the ACT-bound softmax exp) are
            # spread evenly across the projection / output-projection chain
            # stream: quota per chain = remaining-available-steps divided by
            # chains left in the window, capped so the shallow (2-deep) S
            # PSUM pool never head-blocks the PE for long.  ph3 chains are
            # all held back until after the last projections so they fill
            # the PE during the exp-heavy attention tail.
            # pending units in (j, g) order; unit (j, g) unblocks once
            # kqv(j) has emitted its K/Q chains for jt <= g (global kqv
            # chain index 12*j + 2*g + 2).  Late V chains only matter for a
            # unit's last kk-tiles, which pacing naturally emits later;
            # runtime semaphores cover the residual ordering.
            # unit (j, g) unblocks after its c-slice's V chains (window
            # positions 1-4) and the K/Q chains for head pair g (positions
            # 5+2g, 6+2g): global kqv chain count 12j + 6 + 2g.  All its
            # reads are then already emitted - no per-step gating needed.
            punits = [
                (12 * j + 6 + 2 * g, j, unit_steps(j, g))
                for j in range(NJ)
                for g in range(JT)
            ]
            avail = []  # step iterators whose inputs are fully projected
            avj = []  # their j values (for the ph3 gate)

            def emit_steps(n):
                done = 0
                while avail and done < n:
                    if next(avail[0], None) is None:
                        avail.pop(0)
                        avj.pop(0)
                    else:
                        done += 1
                return done

            rem = [0]
            gci = 0
            for cs in range(NJ):
                cgen = kqv_chains(cs)
                nch = 3 * JT  # 12 chains per c-slice
                for ci in range(nch):
                    if rem[0] > 0:
                        want = -(-rem[0] // (nch - ci))  # ceil
                        rem[0] -= emit_steps(min(3, want))
                    next(cgen)
                    gci += 1
                    while punits and punits[0][0] <= gci:
                        _, uj, it = punits.pop(0)
                        avail.append(it)
                        avj.append(uj)
                        rem[0] += (uj + 1) * KPJ + 1
            held = None  # tail of ph3(NJ-2), woven into the final drain
            for j3 in range(NJ - 1):
                # correctness: ph3(j3) needs every unit of q-slice j3 done
                while avail and avj[0] <= j3:
                    if next(avail[0], None) is None:
                        avail.pop(0)
                        avj.pop(0)
                    else:
                        rem[0] -= 1
                cgen = ph3_chains(j3)
                nch = 2 * KPJ if j3 < NJ - 2 else KPJ
                for ci in range(nch):
                    if rem[0] > 0:
                        want = -(-rem[0] // (nch - ci))
                        rem[0] -= emit_steps(min(4, want))
                    next(cgen)
                if j3 == NJ - 2:
                    held = cgen
            # final drain: the last unit's exp-paced steps, with the held
            # ph3 chains giving the PE work while ACT grinds through exp
            while True:
                n = emit_steps(3)
                if held is not None and next(held, None) is None:
                    held = None
                if n == 0 and held is None:
                    break
            for _ in ph3_chains(NJ - 1):
                pass
    return nc


def _split_waits_json(bir_json_bytes):
    """TRN2 TPB instructions have one sync-wait slot and this walrus build
    refuses to split multi-wait instructions, so hoist all but the last wait
    onto preceding wait-only EventSemaphore instructions (same engine,
    executed in order -> semantically identical)."""
    import json

    d = json.loads(bir_json_bytes)
    n = 0
    for fn in d["functions"]:
        for blk in fn["blocks"]:
            out = []
            for inst in blk["instructions"]:
                si = inst.get("sync_info")
                waits = (si or {}).get("on_wait") or []
                if len(waits) > 1:
                    for w in waits[:-1]:
                        n += 1
                        out.append(
                            {
                                "debug": inst.get("debug", 0),
                                "engine": inst["engine"],
                                "ins": [],
                                "name": f"wsplit-{n}",
                                "opcode": "EventSemaphore",
                                "outs": [],
                                "sync_info": {"on_update": [], "on_wait": [w]},
                            }
                        )
                    si["on_wait"] = [waits[-1]]
                out.append(inst)
            blk["instructions"] = out
    return json.dumps(d).encode()


def _striped(a, p=P):
    """[K, N] with K = kt*p + i  ->  contiguous [p, K//p, N]."""
    k, n = a.shape
    return np.ascontiguousarray(a.reshape(k // p, p, n).transpose(1, 0, 2))


def prep_core_inputs(x_b, wq_s, wk_s, wv_s, wo_s):
    """Host-side layout prep for one core. x_b [C,E], w*_s column/row slices."""
    import ml_dtypes

    bf16 = ml_dtypes.bfloat16
    tri = np.triu(np.ones((P, P), dtype=np.float32))  # keep where q >= kk
    tm = np.ascontiguousarray(np.stack([tri, tri], axis=1))  # [P, 2, P]
    return {
        "xT": np.ascontiguousarray(
            _striped(np.ascontiguousarray(x_b.T))
            .reshape(P, E // P, C // CS, CS)
            .transpose(0, 2, 1, 3)
        ).astype(bf16),
        "wq": _striped(wq_s).astype(bf16),
        "wk": _striped(wk_s).astype(bf16),
        "wv": _striped(wv_s).astype(bf16),
        "wo": _striped(wo_s).astype(bf16),
        "tm": tm.astype(bf16),
    }


_module_cache = {}


def kernel(x, W_q, W_k, W_v, W_o):
    from concourse.bass_utils import run_bass_kernel_spmd

    x = np.asarray(x, dtype=np.float32)
    W_q = np.asarray(W_q, dtype=np.float32)
    W_k = np.asarray(W_k, dtype=np.float32)
    W_v = np.asarray(W_v, dtype=np.float32)
    W_o = np.asarray(W_o, dtype=np.float32)

    HD2 = H * D // 2  # columns per head-group (512)
    in_maps = []
    for core in range(NCORES):
        b, hg = core // 2, core % 2
        cols = slice(hg * HD2, (hg + 1) * HD2)
        in_maps.append(
            prep_core_inputs(
                x[b], W_q[:, cols], W_k[:, cols], W_v[:, cols], W_o[cols, :]
            )
        )

    if "nc" not in _module_cache:
        nc = build_module()
        fixed = _split_waits_json(nc.to_json_bytes())
        nc.to_json_bytes = lambda: fixed
        _module_cache["nc"] = nc
    nc = _module_cache["nc"]

    res = run_bass_kernel_spmd(nc, in_maps, core_ids=list(range(NCORES)))
    _module_cache["last_res"] = res
    out = np.empty((B, C, E), dtype=np.float32)
    for b in range(B):
        ya = res.results[2 * b]["y"].reshape(C, E)
        yb = res.results[2 * b + 1]["y"].reshape(C, E)
        out[b] = ya + yb
    return out


if __name__ == "__main__":
    rng = np.random.default_rng(0)
    ins = {
        "x": rng.standard_normal((B, C, E), dtype=np.float32),
        "W_q": rng.standard_normal((E, H * D), dtype=np.float32) * 0.02,
        "W_k": rng.standard_normal((E, H * D), dtype=np.float32) * 0.02,
        "W_v": rng.standard_normal((E, H * D), dtype=np.float32) * 0.02,
        "W_o": rng.standard_normal((H * D, E), dtype=np.float32) * 0.02,
    }
    out = kernel(**ins)
    print("kernel ran, out shape", out.shape, "mean", out.mean())
